# revision 1
# baseline (speedup 1.0000x reference)
"""Causal self-attention (B=2, T=2048, C=1024, H=16) on 8 TRN2 NeuronCores.

Sharding: batch x head-group. Core c handles batch b = c//4 and heads
[4g, 4g+4) with g = c%4. Each core:
  - transposes x[b] on the PE (fp32r) to get xT [C, T]
  - computes Q^T/K^T per head pair in [head-dim, T] layout and V in [T, head-dim]
  - computes scores S^T[k, q] per head with row-packed K=64 matmul pairs
  - softmax via exp on ACT (no max subtraction: scores are O(+-15)) with the
    denominator obtained for free from a ones-row appended to V
  - output projection row-parallel over its 4 heads -> partial y [T, C]
Host sums the 4 partials per batch (row-parallel unshard).

All matmuls run in float32r (tf32-like, ~1.5e-4 rms) with fp32 PSUM accumulate.
"""
import os
import sys

sys.path.insert(0, "/opt/trn_rl_repo")

import numpy as np

# Defensive: bass_utils imports antenv.axon_hooks when trace is requested via
# env (BASS_TRACE); provide a stub if the container lacks it.
try:
    import antenv.axon_hooks  # noqa: F401
except ImportError:
    import types
    import antenv
    _m = types.ModuleType("antenv.axon_hooks")
    _m._HOOK = None
    _m.set_axon_ntff_profile_hook = lambda h: setattr(_m, "_HOOK", h)
    _m.get_axon_ntff_profile_hook = lambda: _m._HOOK
    sys.modules["antenv.axon_hooks"] = _m
    antenv.axon_hooks = _m

import concourse.bass as bass
import concourse.mybir as mybir
import concourse.tile as tile
from concourse import bacc
from concourse import bass_utils
from concourse.masks import make_identity

P = 128
B, T, C = 2, 2048, 1024
H, HD = 16, 64
N_CORES = 8
HEADS_PER_CORE = H // 4          # 4
PAIRS = HEADS_PER_CORE // 2      # 2
TT = T // P                      # 16 t-tiles
CT = C // P                      # 8 c-tiles
QC = 512                         # q-chunk size
NQC = T // QC                    # 4 q-chunks
SCALE = 1.0 / np.sqrt(HD)

F32 = mybir.dt.float32
F32R = mybir.dt.float32r
BF16 = mybir.dt.bfloat16
# attention compute mode:
#   f32r  - everything float32r (rel err ~3.7e-4)
#   mixed - softmax weights + V in bf16, scores f32r (rel err ~1e-3, faster AV)
#   bf16  - Q/K/V/weights all bf16 in attention (rel err ~3e-3, fastest)
ATTN_MODE = os.environ.get("ATTN_MODE", "f32r")
QKT_DT = BF16 if ATTN_MODE == "bf16" else F32R
EST_DT = BF16 if ATTN_MODE in ("mixed", "bf16") else F32R

_NC_CACHE = None
LAST_RESULTS = None


def _build():
    nc = bacc.Bacc("TRN2", target_bir_lowering=False, debug=False,
                   enable_asserts=True, num_devices=1)
    xb = nc.dram_tensor("xb", [T, C], F32, kind="ExternalInput").ap()
    wqk = nc.dram_tensor("wqk", [C, 512], F32, kind="ExternalInput").ap()
    wv = nc.dram_tensor("wv", [C, 256], F32, kind="ExternalInput").ap()
    sel = nc.dram_tensor("sel", [2, P], F32, kind="ExternalInput").ap()
    wout = nc.dram_tensor("wout", [256, C], F32, kind="ExternalInput").ap()
    y = nc.dram_tensor("y", [T, C], F32, kind="ExternalOutput").ap()

    with tile.TileContext(nc) as tc:
        _emit(nc, tc, xb, wqk, wv, wout, sel, y)
    nc.compile()
    return nc


def _emit(nc, tc, xb, wqk, wv, wout, sel, y):
    import contextlib
    with contextlib.ExitStack() as ctx:
        # pools that live for the whole kernel
        consts = ctx.enter_context(tc.tile_pool(name="consts", bufs=1))
        qkt_pool = ctx.enter_context(tc.tile_pool(name="qkt", bufs=1))
        v_pool = ctx.enter_context(tc.tile_pool(name="v", bufs=1))

        # ---------------- constants ----------------
        ident_f32 = consts.tile([P, P], F32)
        make_identity(nc, ident_f32[:])
        ident = consts.tile([P, P], F32R)
        nc.vector.tensor_copy(ident[:], ident_f32[:])

        # tri mask [128, 128]: 1.0 where q >= k (upper incl diag)
        maskm = consts.tile([P, P], F32)
        nc.gpsimd.memset(maskm[:], 0.0)
        nc.gpsimd.affine_select(
            out=maskm[:], in_=maskm[:], compare_op=mybir.AluOpType.is_gt,
            fill=1.0, base=0, pattern=[[-1, P]], channel_multiplier=1)
        mask_c = consts.tile([P, P], EST_DT)
        nc.vector.tensor_copy(mask_c[:], maskm[:])
        maskr = mask_c[:]

        sel_sb = consts.tile([2, P], F32R)
        nc.sync.dma_start(sel_sb[:], sel.bitcast(F32R))

        ones_col_f32 = consts.tile([P, HEADS_PER_CORE], F32)
        nc.gpsimd.memset(ones_col_f32[:], 1.0)

        qkt = [qkt_pool.tile([P, T], QKT_DT, tag=f"qkt{ch}", name=f"qkt{ch}")
               for ch in range(4)]
        v_sb = [v_pool.tile([P, HEADS_PER_CORE, HD + 1], EST_DT, tag=f"v{ti}",
                            name=f"v{ti}") for ti in range(TT)]

        with contextlib.ExitStack() as early:
            w_early = early.enter_context(tc.tile_pool(name="w_early", bufs=1))
            xpool = early.enter_context(tc.tile_pool(name="x", bufs=3))
            xt_pool = early.enter_context(tc.tile_pool(name="xt", bufs=1))
            ps_tr = early.enter_context(
                tc.tile_pool(name="ps_tr", bufs=3, space="PSUM"))
            ps_qk = early.enter_context(
                tc.tile_pool(name="ps_qk", bufs=2, space="PSUM"))
            ps_v = early.enter_context(
                tc.tile_pool(name="ps_v", bufs=2, space="PSUM"))

            # ---------------- phase A: x load + transpose ----------------
            xT = [xt_pool.tile([P, T], F32R, tag=f"xt{ci}", name=f"xt{ci}")
                  for ci in range(CT)]
            for ti in range(TT):
                x_t = xpool.tile([P, C], F32R, tag="x_in")
                nc.sync.dma_start(x_t[:, :C // 2],
                                  xb[ti * P:(ti + 1) * P, :C // 2].bitcast(F32R))
                nc.sync.dma_start(x_t[:, C // 2:],
                                  xb[ti * P:(ti + 1) * P, C // 2:].bitcast(F32R))
                for ci in range(CT):
                    pt = ps_tr.tile([P, P], F32R, tag="tr")
                    nc.tensor.transpose(pt[:], x_t[:, ci * P:(ci + 1) * P], ident[:])
                    if ci % 2 == 0:
                        nc.vector.tensor_copy(xT[ci][:, ti * P:(ti + 1) * P], pt[:])
                    else:
                        nc.scalar.copy(xT[ci][:, ti * P:(ti + 1) * P], pt[:])

            wqk_sb = w_early.tile([P, CT, 512], F32R)
            nc.sync.dma_start(wqk_sb[:],
                              wqk.rearrange("(co p) n -> p co n", p=P).bitcast(F32R))
            wv_sb = w_early.tile([P, CT, 256], F32R)
            nc.sync.dma_start(wv_sb[:],
                              wv.rearrange("(co p) n -> p co n", p=P).bitcast(F32R))

            # ---------------- phase B: Q^T/K^T projections ----------------
            # chunk layout: 0 = pair0 Q (headA|headB), 1 = pair0 K, 2/3 = pair1
            for ch in range(4):
                for tch in range(T // 512):
                    pq = ps_qk.tile([P, 512], F32, tag="qk")
                    for ci in range(CT):
                        nc.tensor.matmul(
                            pq[:], wqk_sb[:, ci, ch * P:(ch + 1) * P],
                            xT[ci][:, tch * 512:(tch + 1) * 512],
                            start=(ci == 0), stop=(ci == CT - 1))
                    nc.scalar.copy(
                        qkt[ch][:, tch * 512:(tch + 1) * 512], pq[:])

            # ---------------- phase C: V (+ones row) ----------------
            for ti in range(TT):
                pv = ps_v.tile([P, 256], F32, tag="v")
                for ci in range(CT):
                    nc.tensor.matmul(
                        pv[:], xT[ci][:, ti * P:(ti + 1) * P], wv_sb[:, ci],
                        start=(ci == 0), stop=(ci == CT - 1))
                nc.scalar.copy(
                    v_sb[ti][:, :, 0:HD],
                    pv[:].rearrange("p (h d) -> p h d", h=HEADS_PER_CORE))
                nc.vector.tensor_copy(v_sb[ti][:, :, HD], ones_col_f32[:])

        with contextlib.ExitStack() as late:
            ot_pool = late.enter_context(tc.tile_pool(name="ot", bufs=1))
            est_pool = late.enter_context(tc.tile_pool(name="est", bufs=4))
            sb_misc = late.enter_context(tc.tile_pool(name="misc", bufs=2))
            w_late = late.enter_context(tc.tile_pool(name="w_late", bufs=1))
            ysb_pool = late.enter_context(tc.tile_pool(name="ysb", bufs=2))
            ps_s = late.enter_context(
                tc.tile_pool(name="ps_s", bufs=2, space="PSUM"))
            ps_o = late.enter_context(
                tc.tile_pool(name="ps_o", bufs=1, space="PSUM"))
            ps_by = late.enter_context(
                tc.tile_pool(name="ps_by", bufs=2, space="PSUM"))

            wout_sb = w_late.tile([P, 2, C], F32R)
            nc.sync.dma_start(wout_sb[:],
                              wout.rearrange("(pr p) n -> p pr n", p=P).bitcast(F32R))

            # ---------------- phase D: attention ----------------
            ot = [ot_pool.tile([P, T], F32R, tag=f"ot{p}", name=f"ot{p}")
                  for p in range(PAIRS)]

            def emit_out_proj(ti):
                ysb = ysb_pool.tile([P, C], F32, tag="y", name="ysb")
                for cc in range(C // 512):
                    py = ps_by.tile([P, 512], F32, tag="by", name="py")
                    for pp in range(PAIRS):
                        nc.tensor.matmul(
                            py[:], ot[pp][:, ti * P:(ti + 1) * P],
                            wout_sb[:, pp, cc * 512:(cc + 1) * 512],
                            start=(pp == 0), stop=(pp == PAIRS - 1))
                    nc.vector.tensor_copy(ysb[:, cc * 512:(cc + 1) * 512], py[:])
                    nc.sync.dma_start(
                        y[ti * P:(ti + 1) * P, cc * 512:(cc + 1) * 512],
                        ysb[:, cc * 512:(cc + 1) * 512])

            dram_tmp = late.enter_context(
                tc.tile_pool(name="dram_tmp", bufs=2, space="DRAM"))
            pending_norm = []

            def emit_norm(p, qc, po):
                # stash raw O into ot, bounce denom rows through DRAM,
                # batched reciprocal, one sel matmul, in-place multiply
                qsl = slice(qc * QC, (qc + 1) * QC)
                stage = sb_misc.tile([HD + 1, 2 * QC], F32, tag="stage",
                                     name="stage")
                for half in range(2):
                    nc.vector.tensor_copy(
                        ot[p][half * HD:(half + 1) * HD, qsl],
                        po[half][0:HD, :])
                    nc.vector.tensor_copy(
                        stage[HD:HD + 1, half * QC:(half + 1) * QC],
                        po[half][HD:HD + 1, :])
                dtmp = dram_tmp.tile([2, QC], F32, name="dtmp")
                nc.sync.dma_start(
                    dtmp[:].rearrange("r n -> (r n)")[None, :],
                    stage[HD:HD + 1, :])
                den = sb_misc.tile([2, QC], F32, tag="den", name="den")
                nc.sync.dma_start(den[:], dtmp[:])

                def finish():
                    recip_st = sb_misc.tile([2, QC], F32R, tag="recip",
                                            name="recip_st")
                    with nc.allow_low_precision(reason="f32r recip"):
                        nc.vector.reciprocal(recip_st[:], den[:])
                    pb = ps_by.tile([P, QC], F32, tag="by", name="pb")
                    nc.tensor.matmul(pb[:], sel_sb[:], recip_st[:],
                                     start=True, stop=True)
                    bcast = sb_misc.tile([P, QC], F32, tag="bcast", name="bcast")
                    nc.vector.tensor_copy(bcast[:], pb[:])
                    nc.vector.tensor_tensor(
                        ot[p][:, qsl], ot[p][:, qsl],
                        bcast[:], mybir.AluOpType.mult)
                    if p == PAIRS - 1:
                        for ti in range(4 * qc, 4 * qc + 4):
                            emit_out_proj(ti)
                pending_norm.append(finish)

            def drain_norm():
                while pending_norm:
                    pending_norm.pop(0)()

            for p in range(PAIRS):
                qt_t, kt_t = qkt[2 * p], qkt[2 * p + 1]
                for qc in range(NQC):
                    kmax = 4 * (qc + 1)
                    qsl = slice(qc * QC, (qc + 1) * QC)
                    po = [ps_o.tile([HD + 1, QC], F32, tag=f"o{h}", name=f"po{h}")
                          for h in range(2)]
                    for kt in range(kmax):
                        r = kt - (kmax - 4)
                        sp = max(r, 0) * P     # valid q-span starts here
                        ksl = slice(kt * P, (kt + 1) * P)
                        qsub = slice(qc * QC + sp, (qc + 1) * QC)
                        ps = ps_s.tile([P, 2 * QC], F32, tag="s")
                        nc.tensor.matmul(ps[:, sp:QC], kt_t[0:HD, ksl],
                                         qt_t[0:HD, qsub],
                                         start=True, stop=True,
                                         tile_position=(0, 0))
                        nc.tensor.matmul(ps[:, QC + sp:], kt_t[HD:, ksl],
                                         qt_t[HD:, qsub],
                                         start=True, stop=True,
                                         tile_position=(HD, 0))
                        est = est_pool.tile([P, 2 * QC], EST_DT, tag="est")
                        if sp <= P:
                            nc.scalar.activation(est[:], ps[:],
                                                 mybir.ActivationFunctionType.Exp,
                                                 scale=SCALE)
                        else:
                            for half in range(2):
                                off = half * QC
                                nc.scalar.activation(
                                    est[:, off + sp:off + QC],
                                    ps[:, off + sp:off + QC],
                                    mybir.ActivationFunctionType.Exp,
                                    scale=SCALE)
                        if r >= 0:
                            for half in range(2):
                                off = half * QC
                                nc.vector.tensor_tensor(
                                    est[:, off + r * P: off + (r + 1) * P],
                                    est[:, off + r * P: off + (r + 1) * P],
                                    maskr[:], mybir.AluOpType.mult)
                        for half in range(2):
                            nc.tensor.matmul(
                                po[half][:, sp:],
                                v_sb[kt][:, 2 * p + half],
                                est[:, half * QC + sp:(half + 1) * QC],
                                start=(kt == 0), stop=(kt == kmax - 1))
                    emit_norm(p, qc, po)
                    while len(pending_norm) >= 2:
                        pending_norm.pop(0)()
            drain_norm()


def _get_nc():
    global _NC_CACHE
    if _NC_CACHE is None:
        _NC_CACHE = _build()
    return _NC_CACHE


def kernel(x, w_qkv, w_out):
    global LAST_RESULTS
    x = np.asarray(x, dtype=np.float32)
    w_qkv = np.asarray(w_qkv, dtype=np.float32)
    w_out = np.asarray(w_out, dtype=np.float32)

    wq, wk, wv = w_qkv[:, 0:C], w_qkv[:, C:2 * C], w_qkv[:, 2 * C:3 * C]

    in_maps = []
    for c in range(N_CORES):
        b, g = c // 4, c % 4
        heads = [4 * g + i for i in range(HEADS_PER_CORE)]
        cols = lambda w, h: w[:, h * HD:(h + 1) * HD]
        wqk_c = np.concatenate([
            cols(wq, heads[0]), cols(wq, heads[1]),
            cols(wk, heads[0]), cols(wk, heads[1]),
            cols(wq, heads[2]), cols(wq, heads[3]),
            cols(wk, heads[2]), cols(wk, heads[3]),
        ], axis=1)
        wv_c = wv[:, heads[0] * HD:(heads[-1] + 1) * HD]
        wout_c = w_out[heads[0] * HD:(heads[-1] + 1) * HD, :]
        sel_np = np.zeros((2, 128), dtype=np.float32)
        sel_np[0, 0:64] = 1.0
        sel_np[1, 64:128] = 1.0
        in_maps.append({
            "xb": np.ascontiguousarray(x[b]),
            "sel": sel_np,
            "wqk": np.ascontiguousarray(wqk_c),
            "wv": np.ascontiguousarray(wv_c),
            "wout": np.ascontiguousarray(wout_c),
        })

    nc = _get_nc()
    res = bass_utils.run_bass_kernel_spmd(
        nc, in_maps, core_ids=list(range(N_CORES)),
        trace=bool(os.environ.get("ATTN_TRACE")))
    LAST_RESULTS = res

    out = np.zeros((B, T, C), dtype=np.float64)
    for c in range(N_CORES):
        out[c // 4] += res.results[c]["y"].astype(np.float64)
    return out.astype(np.float32)



# revision 3
# speedup vs baseline: 1.2216x; 1.2216x over previous
"""Causal self-attention (B=2, T=2048, C=1024, H=16) on 8 TRN2 NeuronCores.

Sharding: batch x head-group. Core c handles batch b = c//4 and heads
[4g, 4g+4) with g = c%4.

v2 changes vs baseline:
  - everything bf16 on-chip (PSUM accumulation stays fp32): halves DMA,
    enables FWL weight loads, 2x DVE on 16-bit ops
  - x is transposed on the HOST -> xT [C, T] DMA'd directly; the whole
    PE-transpose phase A (128 transposes + 128 PSUM->SBUF copies) is gone
  - reciprocal computed on a [128, 8] reshape (DRAM bounce) instead of
    [2, 512] (DVE reciprocal cost is free-dim-driven: 3.3us -> ~60ns)
  - normalization multiplies ot directly by the PSUM broadcast (no bcast
    SBUF copy)
  - PSUM->SBUF copies on DVE (ACT only does exp)

Per core:
  - Q^T/K^T per head pair in [head-dim, T] layout; V in [T, head-dim]
  - scores S^T[k, q] per head with row-packed K=64 matmul pairs
  - softmax via exp on ACT (no max subtraction: scores are O(+-15)); the
    denominator comes free from a ones-column appended to V
  - output projection row-parallel over its 4 heads -> partial y [T, C]
Host sums the 4 partials per batch (row-parallel unshard).
"""
import os
import sys

sys.path.insert(0, "/opt/trn_rl_repo")

import numpy as np
import ml_dtypes

# Defensive: bass_utils imports antenv.axon_hooks when trace is requested via
# env; provide a stub if the container lacks it.
try:
    import antenv.axon_hooks  # noqa: F401
except ImportError:
    import types
    import antenv
    _m = types.ModuleType("antenv.axon_hooks")
    _m._HOOK = None
    _m.set_axon_ntff_profile_hook = lambda h: setattr(_m, "_HOOK", h)
    _m.get_axon_ntff_profile_hook = lambda: _m._HOOK
    sys.modules["antenv.axon_hooks"] = _m
    antenv.axon_hooks = _m

import concourse.bass as bass
import concourse.mybir as mybir
import concourse.tile as tile
from concourse import bacc
from concourse import bass_utils

P = 128
B, T, C = 2, 2048, 1024
H, HD = 16, 64
N_CORES = 8
HEADS_PER_CORE = H // 4          # 4
PAIRS = HEADS_PER_CORE // 2      # 2
TT = T // P                      # 16 t-tiles
CT = C // P                      # 8 c-tiles
QC = 512                         # q-chunk size
NQC = T // QC                    # 4 q-chunks
SCALE = 1.0 / np.sqrt(HD)

F32 = mybir.dt.float32
BF16 = mybir.dt.bfloat16
NP_BF16 = ml_dtypes.bfloat16

_NC_CACHE = None
LAST_RESULTS = None


def _build():
    nc = bacc.Bacc("TRN2", target_bir_lowering=False, debug=False,
                   enable_asserts=True, num_devices=1)
    xt = nc.dram_tensor("xt", [C, T], BF16, kind="ExternalInput").ap()
    wqk = nc.dram_tensor("wqk", [C, 512], BF16, kind="ExternalInput").ap()
    wv = nc.dram_tensor("wv", [C, 256], BF16, kind="ExternalInput").ap()
    sel = nc.dram_tensor("sel", [2, P], BF16, kind="ExternalInput").ap()
    wout = nc.dram_tensor("wout", [256, C], BF16, kind="ExternalInput").ap()
    y = nc.dram_tensor("y", [T, C], BF16, kind="ExternalOutput").ap()

    with tile.TileContext(nc) as tc:
        _emit(nc, tc, xt, wqk, wv, wout, sel, y)
    nc.compile()
    return nc


def _emit(nc, tc, xt, wqk, wv, wout, sel, y):
    import contextlib
    with contextlib.ExitStack() as ctx:
        consts = ctx.enter_context(tc.tile_pool(name="consts", bufs=1))
        qkt_pool = ctx.enter_context(tc.tile_pool(name="qkt", bufs=1))
        v_pool = ctx.enter_context(tc.tile_pool(name="v", bufs=1))

        # ---------------- constants ----------------
        # tri mask [128, 128]: 1.0 where q >= k (upper incl diag)
        maskm = consts.tile([P, P], F32)
        nc.gpsimd.memset(maskm[:], 0.0)
        nc.gpsimd.affine_select(
            out=maskm[:], in_=maskm[:], compare_op=mybir.AluOpType.is_gt,
            fill=1.0, base=0, pattern=[[-1, P]], channel_multiplier=1)
        mask_c = consts.tile([P, P], BF16)
        nc.vector.tensor_copy(mask_c[:], maskm[:])
        maskr = mask_c[:]

        sel_sb = consts.tile([2, P], BF16)
        nc.sync.dma_start(sel_sb[:], sel)

        ones_col = consts.tile([P, HEADS_PER_CORE], BF16)
        nc.gpsimd.memset(ones_col[:], 1.0)

        qkt = [qkt_pool.tile([P, T], BF16, tag=f"qkt{ch}", name=f"qkt{ch}")
               for ch in range(4)]
        v_sb = [v_pool.tile([P, HEADS_PER_CORE, HD + 1], BF16, tag=f"v{ti}",
                            name=f"v{ti}") for ti in range(TT)]

        with contextlib.ExitStack() as early:
            w_early = early.enter_context(tc.tile_pool(name="w_early", bufs=1))
            xt_pool = early.enter_context(tc.tile_pool(name="xt", bufs=1))
            ps_qk = early.enter_context(
                tc.tile_pool(name="ps_qk", bufs=2, space="PSUM"))
            ps_v = early.enter_context(
                tc.tile_pool(name="ps_v", bufs=2, space="PSUM"))

            wqk_sb = w_early.tile([P, CT, 512], BF16)
            nc.sync.dma_start(wqk_sb[:], wqk.rearrange("(co p) n -> p co n", p=P))
            wv_sb = w_early.tile([P, CT, 256], BF16)
            nc.sync.dma_start(wv_sb[:], wv.rearrange("(co p) n -> p co n", p=P))

            # ---------------- xT load (host pre-transposed) ----------------
            xT = [xt_pool.tile([P, T], BF16, tag=f"xt{ci}", name=f"xt{ci}")
                  for ci in range(CT)]
            for tch in range(T // 512):
                tsl = slice(tch * 512, (tch + 1) * 512)
                for ci in range(CT):
                    nc.sync.dma_start(xT[ci][:, tsl],
                                      xt[ci * P:(ci + 1) * P, tsl])

            # ---------------- phase B: Q^T/K^T projections ----------------
            # chunk layout: 0 = pair0 Q (headA|headB), 1 = pair0 K, 2/3 = pair1
            for ch in range(4):
                for tch in range(T // 512):
                    pq = ps_qk.tile([P, 512], F32, tag="qk")
                    for ci in range(CT):
                        nc.tensor.matmul(
                            pq[:], wqk_sb[:, ci, ch * P:(ch + 1) * P],
                            xT[ci][:, tch * 512:(tch + 1) * 512],
                            start=(ci == 0), stop=(ci == CT - 1))
                    nc.vector.tensor_copy(
                        qkt[ch][:, tch * 512:(tch + 1) * 512], pq[:])

            # ---------------- phase C: V (+ones col) ----------------
            for ti in range(TT):
                pv = ps_v.tile([P, 256], F32, tag="v")
                for ci in range(CT):
                    nc.tensor.matmul(
                        pv[:], xT[ci][:, ti * P:(ti + 1) * P], wv_sb[:, ci],
                        start=(ci == 0), stop=(ci == CT - 1))
                nc.vector.tensor_copy(
                    v_sb[ti][:, :, 0:HD],
                    pv[:].rearrange("p (h d) -> p h d", h=HEADS_PER_CORE))
                nc.vector.tensor_copy(v_sb[ti][:, :, HD], ones_col[:])

        with contextlib.ExitStack() as late:
            ot_pool = late.enter_context(tc.tile_pool(name="ot", bufs=1))
            est_pool = late.enter_context(tc.tile_pool(name="est", bufs=4))
            sb_misc = late.enter_context(tc.tile_pool(name="misc", bufs=2))
            w_late = late.enter_context(tc.tile_pool(name="w_late", bufs=1))
            ysb_pool = late.enter_context(tc.tile_pool(name="ysb", bufs=3))
            ps_s = late.enter_context(
                tc.tile_pool(name="ps_s", bufs=2, space="PSUM"))
            ps_o = late.enter_context(
                tc.tile_pool(name="ps_o", bufs=1, space="PSUM"))
            ps_by = late.enter_context(
                tc.tile_pool(name="ps_by", bufs=2, space="PSUM"))

            wout_sb = w_late.tile([P, 2, C], BF16)
            nc.sync.dma_start(wout_sb[:],
                              wout.rearrange("(pr p) n -> p pr n", p=P))

            # ---------------- phase D: attention ----------------
            ot = [ot_pool.tile([P, T], BF16, tag=f"ot{p}", name=f"ot{p}")
                  for p in range(PAIRS)]

            def emit_out_proj(ti):
                for cc in range(C // 512):
                    py = ps_by.tile([P, 512], F32, tag="by", name="py")
                    for pp in range(PAIRS):
                        nc.tensor.matmul(
                            py[:], ot[pp][:, ti * P:(ti + 1) * P],
                            wout_sb[:, pp, cc * 512:(cc + 1) * 512],
                            start=(pp == 0), stop=(pp == PAIRS - 1))
                    ysb = ysb_pool.tile([P, 512], BF16, tag="y", name="ysb")
                    nc.vector.tensor_copy(ysb[:], py[:])
                    nc.sync.dma_start(
                        y[ti * P:(ti + 1) * P, cc * 512:(cc + 1) * 512],
                        ysb[:])

            dram_tmp = late.enter_context(
                tc.tile_pool(name="dram_tmp", bufs=2, space="DRAM"))
            pending_norm = []

            def emit_norm(p, qc, po):
                # stash raw O into ot; bounce denom rows through DRAM into a
                # [128, 8] layout (DVE reciprocal cost is free-dim-driven),
                # reciprocal there, bounce back to [2, QC], one sel matmul,
                # multiply ot in place straight from PSUM.
                qsl = slice(qc * QC, (qc + 1) * QC)
                stage = sb_misc.tile([HD + 1, 2 * QC], F32, tag="stage",
                                     name="stage")
                for half in range(2):
                    nc.vector.tensor_copy(
                        ot[p][half * HD:(half + 1) * HD, qsl],
                        po[half][0:HD, :])
                    nc.vector.tensor_copy(
                        stage[HD:HD + 1, half * QC:(half + 1) * QC],
                        po[half][HD:HD + 1, :])
                dtmp = dram_tmp.tile([2, QC], F32, name="dtmp")
                nc.sync.dma_start(
                    dtmp[:].rearrange("r n -> (r n)")[None, :],
                    stage[HD:HD + 1, :])
                den128 = sb_misc.tile([P, 8], F32, tag="den", name="den128")
                nc.sync.dma_start(
                    den128[:], dtmp[:].rearrange("r (g f) -> (r g) f", f=8))

                def finish():
                    recip128 = sb_misc.tile([P, 8], BF16, tag="recip",
                                            name="recip128")
                    with nc.allow_low_precision(reason="bf16 recip"):
                        nc.vector.reciprocal(recip128[:], den128[:])
                    dtmp2 = dram_tmp.tile([2, QC], BF16, name="dtmp2")
                    nc.sync.dma_start(
                        dtmp2[:].rearrange("r (g f) -> (r g) f", f=8),
                        recip128[:])
                    recip2 = sb_misc.tile([2, QC], BF16, tag="recip2",
                                          name="recip2")
                    nc.sync.dma_start(recip2[:], dtmp2[:])
                    pb = ps_by.tile([P, QC], F32, tag="by", name="pb")
                    nc.tensor.matmul(pb[:], sel_sb[:], recip2[:],
                                     start=True, stop=True)
                    nc.vector.tensor_tensor(
                        ot[p][:, qsl], ot[p][:, qsl],
                        pb[:], mybir.AluOpType.mult)
                    if p == PAIRS - 1:
                        for ti in range(4 * qc, 4 * qc + 4):
                            emit_out_proj(ti)
                pending_norm.append(finish)

            def drain_norm():
                while pending_norm:
                    pending_norm.pop(0)()

            for p in range(PAIRS):
                qt_t, kt_t = qkt[2 * p], qkt[2 * p + 1]
                for qc in range(NQC):
                    kmax = 4 * (qc + 1)
                    po = [ps_o.tile([HD + 1, QC], F32, tag=f"o{h}", name=f"po{h}")
                          for h in range(2)]
                    for kt in range(kmax):
                        r = kt - (kmax - 4)
                        sp = max(r, 0) * P     # valid q-span starts here
                        ksl = slice(kt * P, (kt + 1) * P)
                        qsub = slice(qc * QC + sp, (qc + 1) * QC)
                        ps = ps_s.tile([P, 2 * QC], F32, tag="s")
                        nc.tensor.matmul(ps[:, sp:QC], kt_t[0:HD, ksl],
                                         qt_t[0:HD, qsub],
                                         start=True, stop=True,
                                         tile_position=(0, 0))
                        nc.tensor.matmul(ps[:, QC + sp:], kt_t[HD:, ksl],
                                         qt_t[HD:, qsub],
                                         start=True, stop=True,
                                         tile_position=(HD, 0))
                        est = est_pool.tile([P, 2 * QC], BF16, tag="est")
                        if sp <= P:
                            nc.scalar.activation(est[:], ps[:],
                                                 mybir.ActivationFunctionType.Exp,
                                                 scale=SCALE)
                        else:
                            for half in range(2):
                                off = half * QC
                                nc.scalar.activation(
                                    est[:, off + sp:off + QC],
                                    ps[:, off + sp:off + QC],
                                    mybir.ActivationFunctionType.Exp,
                                    scale=SCALE)
                        if r >= 0:
                            for half in range(2):
                                off = half * QC
                                nc.vector.tensor_tensor(
                                    est[:, off + r * P: off + (r + 1) * P],
                                    est[:, off + r * P: off + (r + 1) * P],
                                    maskr[:], mybir.AluOpType.mult)
                        for half in range(2):
                            nc.tensor.matmul(
                                po[half][:, sp:],
                                v_sb[kt][:, 2 * p + half],
                                est[:, half * QC + sp:(half + 1) * QC],
                                start=(kt == 0), stop=(kt == kmax - 1))
                    emit_norm(p, qc, po)
                    while len(pending_norm) >= 2:
                        pending_norm.pop(0)()
            drain_norm()


def _get_nc():
    global _NC_CACHE
    if _NC_CACHE is None:
        _NC_CACHE = _build()
    return _NC_CACHE


def kernel(x, w_qkv, w_out):
    global LAST_RESULTS
    x = np.asarray(x, dtype=np.float32)
    w_qkv = np.asarray(w_qkv, dtype=np.float32)
    w_out = np.asarray(w_out, dtype=np.float32)

    wq, wk, wv = w_qkv[:, 0:C], w_qkv[:, C:2 * C], w_qkv[:, 2 * C:3 * C]

    xt_all = [np.ascontiguousarray(x[b].T).astype(NP_BF16) for b in range(B)]

    in_maps = []
    for c in range(N_CORES):
        b, g = c // 4, c % 4
        heads = [4 * g + i for i in range(HEADS_PER_CORE)]
        cols = lambda w, h: w[:, h * HD:(h + 1) * HD]
        wqk_c = np.concatenate([
            cols(wq, heads[0]), cols(wq, heads[1]),
            cols(wk, heads[0]), cols(wk, heads[1]),
            cols(wq, heads[2]), cols(wq, heads[3]),
            cols(wk, heads[2]), cols(wk, heads[3]),
        ], axis=1)
        wv_c = wv[:, heads[0] * HD:(heads[-1] + 1) * HD]
        wout_c = w_out[heads[0] * HD:(heads[-1] + 1) * HD, :]
        sel_np = np.zeros((2, 128), dtype=NP_BF16)
        sel_np[0, 0:64] = 1.0
        sel_np[1, 64:128] = 1.0
        in_maps.append({
            "xt": xt_all[b],
            "sel": sel_np,
            "wqk": np.ascontiguousarray(wqk_c).astype(NP_BF16),
            "wv": np.ascontiguousarray(wv_c).astype(NP_BF16),
            "wout": np.ascontiguousarray(wout_c).astype(NP_BF16),
        })

    nc = _get_nc()
    res = bass_utils.run_bass_kernel_spmd(
        nc, in_maps, core_ids=list(range(N_CORES)),
        trace=bool(os.environ.get("ATTN_TRACE")))
    LAST_RESULTS = res

    out = np.zeros((B, T, C), dtype=np.float64)
    for c in range(N_CORES):
        out[c // 4] += res.results[c]["y"].astype(np.float64)
    return out.astype(np.float32)


# revision 10
# speedup vs baseline: 1.2631x; 1.0340x over previous
"""Causal self-attention (B=2, T=2048, C=1024, H=16) on 8 TRN2 NeuronCores.

Sharding: batch x head-group. Core c handles batch b = c//4 and heads
[4g, 4g+4) with g = c%4.

v3 structure (all bf16 on-chip, fp32 PSUM accumulate):
  - host pre-transposes x -> xT [C, T] and converts inputs to bf16
  - warmup matmuls on memset tiles fill the initial DMA wait and get the
    PE clock to full rate before real work lands
  - emission order B-pair0 -> C -> D-pair0 -> B-pair1 -> D-pair1 lets the
    scheduler drop pair1 projection matmuls into D-pair0's exp stalls
  - PSUM->SBUF copies go to ACT during B/C (ACT idle there), DVE during D
  - softmax denominators bounce through DRAM into [128, 8] for the
    reciprocal (DVE reciprocal cost is free-dim-driven)
  - ones column of V via gpsimd memset
Host sums the 4 partial y's per batch (row-parallel unshard).
"""
import os
import sys

sys.path.insert(0, "/opt/trn_rl_repo")

import numpy as np
import ml_dtypes

try:
    import antenv.axon_hooks  # noqa: F401
except ImportError:
    import types
    import antenv
    _m = types.ModuleType("antenv.axon_hooks")
    _m._HOOK = None
    _m.set_axon_ntff_profile_hook = lambda h: setattr(_m, "_HOOK", h)
    _m.get_axon_ntff_profile_hook = lambda: _m._HOOK
    sys.modules["antenv.axon_hooks"] = _m
    antenv.axon_hooks = _m

import concourse.bass as bass
import concourse.mybir as mybir
import concourse.tile as tile
from concourse import bacc
from concourse import bass_utils

P = 128
B, T, C = 2, 2048, 1024
H, HD = 16, 64
N_CORES = 8
HEADS_PER_CORE = H // 4          # 4
PAIRS = HEADS_PER_CORE // 2      # 2
TT = T // P                      # 16 t-tiles
CT = C // P                      # 8 c-tiles
QC = 512                         # q-chunk size
NQC = T // QC                    # 4 q-chunks
SCALE = 1.0 / np.sqrt(HD)
N_WARMUP = int(os.environ.get("ATTN_WARMUP", "20"))

F32 = mybir.dt.float32
BF16 = mybir.dt.bfloat16
NP_BF16 = ml_dtypes.bfloat16

_NC_CACHE = None
LAST_RESULTS = None


def _build():
    nc = bacc.Bacc("TRN2", target_bir_lowering=False, debug=False,
                   enable_asserts=True, num_devices=1)
    xt = nc.dram_tensor("xt", [C, T], BF16, kind="ExternalInput").ap()
    wqk = nc.dram_tensor("wqk", [C, 512], BF16, kind="ExternalInput").ap()
    wv = nc.dram_tensor("wv", [C, 256], BF16, kind="ExternalInput").ap()
    sel = nc.dram_tensor("sel", [2, P], BF16, kind="ExternalInput").ap()
    wout = nc.dram_tensor("wout", [256, C], BF16, kind="ExternalInput").ap()
    y = nc.dram_tensor("y", [T, C], BF16, kind="ExternalOutput").ap()

    with tile.TileContext(nc) as tc:
        _emit(nc, tc, xt, wqk, wv, wout, sel, y)
    nc.compile()
    return nc


def _emit(nc, tc, xt, wqk, wv, wout, sel, y):
    import contextlib
    with contextlib.ExitStack() as ctx:
        ep = ctx.enter_context
        consts = ep(tc.tile_pool(name="consts", bufs=1))
        qkt_pool = ep(tc.tile_pool(name="qkt", bufs=1))
        v_pool = ep(tc.tile_pool(name="v", bufs=1))
        w_pool = ep(tc.tile_pool(name="w", bufs=1))
        xt_pool = ep(tc.tile_pool(name="xt", bufs=1))
        ot_pool = ep(tc.tile_pool(name="ot", bufs=1))
        est_pool = ep(tc.tile_pool(name="est", bufs=4))
        sb_misc = ep(tc.tile_pool(name="misc", bufs=2))
        ysb_pool = ep(tc.tile_pool(name="ysb", bufs=3))
        dram_tmp = ep(tc.tile_pool(name="dram_tmp", bufs=2, space="DRAM"))
        early = contextlib.ExitStack()
        ps_qk = early.enter_context(tc.tile_pool(name="ps_qk", bufs=2,
                                                 space="PSUM"))
        ps_v = early.enter_context(tc.tile_pool(name="ps_v", bufs=2,
                                                space="PSUM"))

        # ---------------- constants (no DMA deps) ----------------
        maskm = consts.tile([P, P], F32)
        nc.gpsimd.memset(maskm[:], 0.0)
        nc.gpsimd.affine_select(
            out=maskm[:], in_=maskm[:], compare_op=mybir.AluOpType.is_gt,
            fill=1.0, base=0, pattern=[[-1, P]], channel_multiplier=1)
        mask_c = consts.tile([P, P], BF16)
        nc.vector.tensor_copy(mask_c[:], maskm[:])
        maskr = mask_c[:]

        warm_a = consts.tile([P, P], BF16)
        nc.gpsimd.memset(warm_a[:], 0.125)
        warm_b = consts.tile([P, 512], BF16)
        nc.gpsimd.memset(warm_b[:], 0.125)

        sel_sb = consts.tile([2, P], BF16)
        nc.sync.dma_start(sel_sb[:], sel)

        # ---------------- DMAs, dependency-order ----------------
        wqk_sb = w_pool.tile([P, CT, 512], BF16)
        nc.sync.dma_start(wqk_sb[:], wqk.rearrange("(co p) n -> p co n", p=P))

        xT = [xt_pool.tile([P, T], BF16, tag=f"xt{ci}", name=f"xt{ci}")
              for ci in range(CT)]

        def load_xt(tch):
            tsl = slice(tch * 512, (tch + 1) * 512)
            for ci in range(CT):
                nc.sync.dma_start(xT[ci][:, tsl], xt[ci * P:(ci + 1) * P, tsl])

        load_xt(0)
        wv_sb = w_pool.tile([P, CT, 256], BF16)
        nc.sync.dma_start(wv_sb[:], wv.rearrange("(co p) n -> p co n", p=P))
        for tch in range(1, T // 512):
            load_xt(tch)
        wout_sb = w_pool.tile([P, 2, C], BF16)
        nc.sync.dma_start(wout_sb[:], wout.rearrange("(pr p) n -> p pr n", p=P))

        # ---------------- PE warmup during the DMA ramp ----------------
        pw = ps_qk.tile([P, 512], F32, tag="qk", name="pw")
        for i in range(N_WARMUP):
            nc.tensor.matmul(pw[:], warm_a[:], warm_b[:],
                             start=(i == 0), stop=(i == N_WARMUP - 1))
        # keep the warmup live past DCE with a tiny DRAM write
        wsink = sb_misc.tile([1, 8], F32, tag="wsink", name="wsink")
        nc.vector.tensor_copy(wsink[:], pw[0:1, 0:8])
        dwarm = dram_tmp.tile([1, 8], F32, name="dwarm")
        nc.sync.dma_start(dwarm[:], wsink[:])

        qkt = [qkt_pool.tile([P, T], BF16, tag=f"qkt{ch}", name=f"qkt{ch}")
               for ch in range(4)]
        v_sb = [v_pool.tile([P, HEADS_PER_CORE, HD + 1], BF16, tag=f"v{ti}",
                            name=f"v{ti}") for ti in range(TT)]

        # ---------------- phase B: Q^T/K^T projections ----------------
        # chunk layout: 0 = pair0 Q (headA|headB), 1 = pair0 K, 2/3 = pair1
        def emit_b(ch, pool, tag):
            for tch in range(T // 512):
                pq = pool.tile([P, 512], F32, tag=tag)
                for ci in range(CT):
                    nc.tensor.matmul(
                        pq[:], wqk_sb[:, ci, ch * P:(ch + 1) * P],
                        xT[ci][:, tch * 512:(tch + 1) * 512],
                        start=(ci == 0), stop=(ci == CT - 1))
                # ACT is idle outside attention; DVE carries phase-D work
                nc.scalar.copy(qkt[ch][:, tch * 512:(tch + 1) * 512], pq[:])

        emit_b(0, ps_qk, "qk")
        emit_b(1, ps_qk, "qk")

        # ---------------- phase C: V (+ones col) ----------------
        for ti in range(TT):
            pv = ps_v.tile([P, 256], F32, tag="v")
            for ci in range(CT):
                nc.tensor.matmul(
                    pv[:], xT[ci][:, ti * P:(ti + 1) * P], wv_sb[:, ci],
                    start=(ci == 0), stop=(ci == CT - 1))
            nc.scalar.copy(
                v_sb[ti][:, :, 0:HD],
                pv[:].rearrange("p (h d) -> p h d", h=HEADS_PER_CORE))
            nc.gpsimd.memset(v_sb[ti][:, :, HD], 1.0)

        # close B/C PSUM pools so phase D pools get their banks
        early.close()
        ps_s = ep(tc.tile_pool(name="ps_s", bufs=2, space="PSUM"))
        ps_o = ep(tc.tile_pool(name="ps_o", bufs=1, space="PSUM"))
        ps_by = ep(tc.tile_pool(name="ps_by", bufs=2, space="PSUM"))

        # ---------------- phase D: attention ----------------
        ot = [ot_pool.tile([P, T], BF16, tag=f"ot{p}", name=f"ot{p}")
              for p in range(PAIRS)]

        def emit_out_proj(ti):
            for cc in range(C // 512):
                py = ps_by.tile([P, 512], F32, tag="by", name="py")
                for pp in range(PAIRS):
                    nc.tensor.matmul(
                        py[:], ot[pp][:, ti * P:(ti + 1) * P],
                        wout_sb[:, pp, cc * 512:(cc + 1) * 512],
                        start=(pp == 0), stop=(pp == PAIRS - 1))
                ysb = ysb_pool.tile([P, 512], BF16, tag="y", name="ysb")
                nc.vector.tensor_copy(ysb[:], py[:])
                nc.sync.dma_start(
                    y[ti * P:(ti + 1) * P, cc * 512:(cc + 1) * 512],
                    ysb[:])

        pending_norm = []

        def emit_norm(p, qc, po):
            # stash raw O into ot; bounce denom rows through DRAM into a
            # [128, 8] layout, reciprocal there, bounce back to [2, QC],
            # one sel matmul, multiply ot in place straight from PSUM.
            last = (p == PAIRS - 1 and qc == NQC - 1)
            qsl = slice(qc * QC, (qc + 1) * QC)
            stage = sb_misc.tile([HD + 1, 2 * QC], F32, tag="stage",
                                 name="stage")
            for half in range(2):
                nc.vector.tensor_copy(
                    ot[p][half * HD:(half + 1) * HD, qsl],
                    po[half][0:HD, :])
                nc.vector.tensor_copy(
                    stage[HD:HD + 1, half * QC:(half + 1) * QC],
                    po[half][HD:HD + 1, :])
            dtmp = dram_tmp.tile([2, QC], F32, name="dtmp")
            nc.sync.dma_start(
                dtmp[:].rearrange("r n -> (r n)")[None, :],
                stage[HD:HD + 1, :])
            if last:
                # tail path: one bounce fewer, direct [2, QC] reciprocal
                den2 = sb_misc.tile([2, QC], F32, tag="den2", name="den2")
                nc.sync.dma_start(den2[:], dtmp[:])
            else:
                den128 = sb_misc.tile([P, 8], F32, tag="den", name="den128")
                nc.sync.dma_start(
                    den128[:], dtmp[:].rearrange("r (g f) -> (r g) f", f=8))

            def finish():
                recip2 = sb_misc.tile([2, QC], BF16, tag="recip2",
                                      name="recip2")
                if last:
                    with nc.allow_low_precision(reason="bf16 recip"):
                        nc.vector.reciprocal(recip2[:], den2[:])
                else:
                    recip128 = sb_misc.tile([P, 8], BF16, tag="recip",
                                            name="recip128")
                    with nc.allow_low_precision(reason="bf16 recip"):
                        nc.vector.reciprocal(recip128[:], den128[:])
                    dtmp2 = dram_tmp.tile([2, QC], BF16, name="dtmp2")
                    nc.sync.dma_start(
                        dtmp2[:].rearrange("r (g f) -> (r g) f", f=8),
                        recip128[:])
                    nc.sync.dma_start(recip2[:], dtmp2[:])
                pb = ps_by.tile([P, QC], F32, tag="by", name="pb")
                nc.tensor.matmul(pb[:], sel_sb[:], recip2[:],
                                 start=True, stop=True)
                nc.vector.tensor_tensor(
                    ot[p][:, qsl], ot[p][:, qsl],
                    pb[:], mybir.AluOpType.mult)
                if p == PAIRS - 1:
                    for ti in range(4 * qc, 4 * qc + 4):
                        emit_out_proj(ti)
            pending_norm.append(finish)

        def drain_norm():
            while pending_norm:
                pending_norm.pop(0)()

        def emit_d(p):
            qt_t, kt_t = qkt[2 * p], qkt[2 * p + 1]
            for qc in range(NQC):
                kmax = 4 * (qc + 1)
                po = [ps_o.tile([HD + 1, QC], F32, tag=f"o{h}", name=f"po{h}")
                      for h in range(2)]
                for kt in range(kmax):
                    r = kt - (kmax - 4)
                    sp = max(r, 0) * P     # valid q-span starts here
                    ksl = slice(kt * P, (kt + 1) * P)
                    qsub = slice(qc * QC + sp, (qc + 1) * QC)
                    ps = ps_s.tile([P, 2 * QC], F32, tag="s")
                    nc.tensor.matmul(ps[:, sp:QC], kt_t[0:HD, ksl],
                                     qt_t[0:HD, qsub],
                                     start=True, stop=True,
                                     tile_position=(0, 0))
                    nc.tensor.matmul(ps[:, QC + sp:], kt_t[HD:, ksl],
                                     qt_t[HD:, qsub],
                                     start=True, stop=True,
                                     tile_position=(HD, 0))
                    est = est_pool.tile([P, 2 * QC], BF16, tag="est")
                    if sp <= P:
                        nc.scalar.activation(est[:], ps[:],
                                             mybir.ActivationFunctionType.Exp,
                                             scale=SCALE)
                    else:
                        for half in range(2):
                            off = half * QC
                            nc.scalar.activation(
                                est[:, off + sp:off + QC],
                                ps[:, off + sp:off + QC],
                                mybir.ActivationFunctionType.Exp,
                                scale=SCALE)
                    if r >= 0:
                        for half in range(2):
                            off = half * QC
                            nc.vector.tensor_tensor(
                                est[:, off + r * P: off + (r + 1) * P],
                                est[:, off + r * P: off + (r + 1) * P],
                                maskr[:], mybir.AluOpType.mult)
                    for half in range(2):
                        nc.tensor.matmul(
                            po[half][:, sp:],
                            v_sb[kt][:, 2 * p + half],
                            est[:, half * QC + sp:(half + 1) * QC],
                            start=(kt == 0), stop=(kt == kmax - 1))
                emit_norm(p, qc, po)
                while len(pending_norm) >= 2:
                    pending_norm.pop(0)()

        emit_d(0)
        emit_b(2, ps_by, "by")
        emit_b(3, ps_by, "by")
        emit_d(1)
        drain_norm()


def _get_nc():
    global _NC_CACHE
    if _NC_CACHE is None:
        _NC_CACHE = _build()
    return _NC_CACHE


def kernel(x, w_qkv, w_out):
    global LAST_RESULTS
    x = np.asarray(x, dtype=np.float32)
    w_qkv = np.asarray(w_qkv, dtype=np.float32)
    w_out = np.asarray(w_out, dtype=np.float32)

    wq, wk, wv = w_qkv[:, 0:C], w_qkv[:, C:2 * C], w_qkv[:, 2 * C:3 * C]

    xt_all = [np.ascontiguousarray(x[b].T).astype(NP_BF16) for b in range(B)]

    in_maps = []
    for c in range(N_CORES):
        b, g = c // 4, c % 4
        heads = [4 * g + i for i in range(HEADS_PER_CORE)]
        cols = lambda w, h: w[:, h * HD:(h + 1) * HD]
        wqk_c = np.concatenate([
            cols(wq, heads[0]), cols(wq, heads[1]),
            cols(wk, heads[0]), cols(wk, heads[1]),
            cols(wq, heads[2]), cols(wq, heads[3]),
            cols(wk, heads[2]), cols(wk, heads[3]),
        ], axis=1)
        wv_c = wv[:, heads[0] * HD:(heads[-1] + 1) * HD]
        wout_c = w_out[heads[0] * HD:(heads[-1] + 1) * HD, :]
        sel_np = np.zeros((2, 128), dtype=NP_BF16)
        sel_np[0, 0:64] = 1.0
        sel_np[1, 64:128] = 1.0
        in_maps.append({
            "xt": xt_all[b],
            "sel": sel_np,
            "wqk": np.ascontiguousarray(wqk_c).astype(NP_BF16),
            "wv": np.ascontiguousarray(wv_c).astype(NP_BF16),
            "wout": np.ascontiguousarray(wout_c).astype(NP_BF16),
        })

    nc = _get_nc()
    res = bass_utils.run_bass_kernel_spmd(
        nc, in_maps, core_ids=list(range(N_CORES)),
        trace=bool(os.environ.get("ATTN_TRACE")))
    LAST_RESULTS = res

    out = np.zeros((B, T, C), dtype=np.float64)
    for c in range(N_CORES):
        out[c // 4] += res.results[c]["y"].astype(np.float64)
    return out.astype(np.float32)
